# revision 6
# baseline (speedup 1.0000x reference)
"""Sparse-attention Trainium2 kernel (nn_Attention_81398220193933).

Strategy (tensor-parallel over heads, 2 heads per NeuronCore):
  - Host pre-lays-out per-core tensors:
      qT  [B, 128, S]  bf16 : rows 0:64 = headA Q^T / sqrt(dh), rows 64:128 = headB
      kT  [B, 128, S]  bf16 : same for K^T (only the first kb*128 keys are loaded)
      vE  [B, 128, KT, 130] bf16 : per k-tile t, partition p = key position t*128+p,
           cols [0:64]=V_A*emb, [64]=emb, [65:129]=V_B*emb, [129]=emb
           where emb[b,k] = exp(bias[k]) * (k < seq_len[b]) (all-valid if seq_len==0).
    Folding the additive key bias + mask multiplicatively into V makes the
    softmax mask/bias free on-device and lets fully-masked k-tiles be skipped.
  - Device, per batch b and key-tile t (kb = ceil(seq_len/128) tiles):
      scores^T [k=128, q=1024] f32 = K_tile^T.T @ Q^T for both heads; the two
          64-contraction-row matmuls are issued back-to-back at base partitions
          0/64 so they run concurrently on separate PE-array row-halves.
      W^T = exp(scores^T): alternates between ScalarE (exact LUT exp -> bf16)
          and VectorE (one-instruction Schraudolph fast-exp: int16(x*128*log2e
          + magic), bitcast to bf16; ~+-2% weight error) to halve the exp wall
          - the single biggest cost in this kernel.
      out[q,65] += W^T_chunk.T @ V_tile accumulated over t in PSUM; column 64
          accumulates the softmax denominator (via the emb column of vE).
    Epilogue per (batch, head): one batched reciprocal + one tensor_tensor
    multiply (PSUM -> SBUF bf16) + DMA; bf16 output is upcast on host.
  - Softmax max-subtraction is unnecessary: logits are O(+-6) and masked keys
    contribute exactly zero through emb; a fully-masked row degenerates to
    softmax over all keys exactly like the jax reference (the -1e12 shift
    cancels there).
  - PSUM: 2 score slots (2 banks each) + 2 head accumulators (2 banks each).
"""

import numpy as np
import ml_dtypes

import concourse.bass as bass
import concourse.mybir as mybir
import concourse.tile as tile
from concourse import bacc
from concourse.bass_utils import run_bass_kernel_spmd

B = 8
S = 1024
UNITS = 1024
H = 16
DH = 64
N_CORES = 8
KT = S // 128  # max key tiles per batch

BF16 = mybir.dt.bfloat16
F32 = mybir.dt.float32
I16 = mybir.dt.int16

# fast-exp: i16 = trunc/round(x * 128*log2e + (16256 - C)); bits viewed as bf16
EXP_SCALE = 128.0 * 1.4426950408889634
EXP_OFF = 16256.0 - 5.25
# engine cost model (us) used only for load balancing exp tiles
ACT_COST = 1.147
DVE_COST = 1.192
EPI_COST = 0.85


def _build_nc(kbs):
    """Build the SPMD Bass program. kbs: per-batch number of 128-key tiles."""
    nc = bacc.Bacc("TRN2", target_bir_lowering=False, debug=False,
                   num_devices=N_CORES)
    qT = nc.dram_tensor("qt", [B, 128, S], BF16, kind="ExternalInput").ap()
    kT = nc.dram_tensor("kt", [B, 128, S], BF16, kind="ExternalInput").ap()
    vE = nc.dram_tensor("vt", [B, 128, KT, 130], BF16, kind="ExternalInput").ap()
    o = nc.dram_tensor("o", [B, 128, 2, KT, 64], BF16, kind="ExternalOutput").ap()

    with tile.TileContext(nc) as tc:
        with (
            tc.tile_pool(name="qk", bufs=2) as qk_pool,
            tc.tile_pool(name="v", bufs=2) as v_pool,
            tc.tile_pool(name="w", bufs=72) as w_pool,
            tc.tile_pool(name="ot", bufs=4) as o_pool,
            tc.tile_pool(name="rc", bufs=8) as r_pool,
            tc.tile_pool(name="sc", bufs=2, space="PSUM") as sc_pool,
            tc.tile_pool(name="acc", bufs=2, space="PSUM") as acc_pool,
        ):
            bal = {"act": 0.0, "dve": 0.0}

            def emit_exp(wt_parent, sc):
                """exp of one [128, S] score tile on the less-loaded engine.
                Returns the bf16-viewed AP of the W tile."""
                if bal["act"] <= bal["dve"]:
                    bal["act"] += ACT_COST
                    wt = w_pool.tile([128, S], BF16, tag="w", name="w")
                    nc.scalar.activation(wt[:], sc[:],
                                         mybir.ActivationFunctionType.Exp)
                    return wt
                bal["dve"] += DVE_COST
                wt = w_pool.tile([128, S], I16, tag="w", name="w")
                nc.vector.tensor_scalar(
                    wt[:], sc[:], EXP_SCALE, EXP_OFF,
                    mybir.AluOpType.mult, mybir.AluOpType.add)
                return wt

            def w_ap(wt):
                ap = wt[:]
                return ap.bitcast(BF16) if wt.tensor.dtype == I16 else ap

            def emit_av(p, g):
                """Emit one A-V accumulation group. Groups 0-7 = head A,
                8-15 = head B; group j output goes to acc[:, j//4,
                (j%4)*65 : (j%4)*65+65] (one PSUM bank per 4 groups)."""
                h, j = divmod(g, 8)
                if j == 0:
                    p["acc"][h] = acc_pool.tile(
                        [128, 2, 512], F32, tag="acc",
                        name=f"acc{p['b']}_{h}")
                grp = p["acc"][h]
                c0 = (j % 4) * 65
                for t in range(p["kb"]):
                    nc.tensor.matmul(
                        grp[:, j // 4, c0:c0 + 65],
                        lhsT=w_ap(p["wts"][h][t])[:, j * 128:(j + 1) * 128],
                        rhs=p["vt"][:, t, h * 65:h * 65 + 65],
                        start=(t == 0), stop=(t == p["kb"] - 1),
                    )
                if j == 7:
                    epilogue(p, h)

            def epilogue(p, h):
                """Divide head h by its denominators and store (bf16)."""
                bal["dve"] += EPI_COST
                acc = p["acc"][h]
                rc = r_pool.tile([128, 2, 4, 1], F32, tag="rc", name="rc")
                den = bass.AP(tensor=acc.tensor, offset=acc.offset + 64,
                              ap=[acc.ap[0], [512, 2], [65, 4], [1, 1]])
                nc.vector.reciprocal(rc[:], den)
                num = bass.AP(tensor=acc.tensor, offset=acc.offset,
                              ap=[acc.ap[0], [512, 2], [65, 4], [1, 64]])
                rc_b = bass.AP(tensor=rc.tensor, offset=rc.offset,
                               ap=[rc.ap[0], rc.ap[1], rc.ap[2], [0, 64]])
                ot = o_pool.tile([128, 2, 4, 64], BF16, tag="ot", name="ot")
                nc.vector.tensor_tensor(ot[:], num, rc_b, mybir.AluOpType.mult)
                nc.sync.dma_start(out=o[p["b"], :, h], in_=ot[:])

            # Preload the exp table-set (~2.7us) while the first DMAs fly.
            wexp = qk_pool.tile([1, 8], F32, tag="wexp", name="wexp", bufs=1)
            nc.vector.memset(wexp[:], 0.0)
            nc.scalar.activation(wexp[:], wexp[:],
                                 mybir.ActivationFunctionType.Exp)
            # HAM keep-warm: the PE clock-gate only delivers 2.4 GHz while the
            # PE looks busy; idle windows re-throttle it to 1.2 GHz. Junk
            # matmuls write into score slots right before their real QK
            # overwrites them (start=True clears the bank), so they need no
            # extra PSUM bank and land exactly in the PE's natural bubbles.
            wu = qk_pool.tile([128, 640], BF16, tag="wu", name="wu", bufs=1)
            nc.vector.memset(wu[:], 0.0)

            def keep_warm(sc_ap, n):
                for _ in range(n):
                    nc.tensor.matmul(sc_ap, lhsT=wu[:, 0:128],
                                     rhs=wu[:, 128:640],
                                     start=True, stop=True,
                                     skip_group_check=True)

            # Load every batch's inputs up front (fits easily in SBUF) so no
            # QK phase ever waits on DMA. First batch small (warms up on real
            # work at low cost), then largest-first, smallest last (short tail
            # after the final exp).
            srt = sorted(range(B), key=lambda i: -kbs[i])
            order = [srt[-2]] + srt[:-2] + [srt[-1]]
            qts, kts, vts = {}, {}, {}
            for b in order:
                qts[b] = qk_pool.tile([128, S], BF16, tag=f"qt{b}",
                                      name=f"qt{b}", bufs=1)
                nc.sync.dma_start(out=qts[b][:], in_=qT[b])
                kts[b] = qk_pool.tile([128, kbs[b] * 128], BF16, tag=f"kt{b}",
                                      name=f"kt{b}", bufs=1)
                nc.sync.dma_start(out=kts[b][:], in_=kT[b, :, :kbs[b] * 128])
            for b in order:
                vts[b] = v_pool.tile([128, kbs[b], 130], BF16, tag=f"vt{b}",
                                     name=f"vt{b}", bufs=1)
                nc.sync.dma_start(out=vts[b][:], in_=vE[b, :, :kbs[b], :])

            # Startup burst: warm the PE during the initial DMA wait.
            scw = sc_pool.tile([128, S], F32, tag="sc", name="scwarm")
            keep_warm(scw[:, 0:512], 3)
            keep_warm(scw[:, 512:1024], 3)

            # Global step stream: one step per (batch, key-tile). A-V groups
            # of finished batches queue up and drip out between steps, so
            # batch boundaries never pile PE work in front of the next QK.
            avq = []  # (batch record, group) FIFO
            total_steps = sum(kbs)
            step_no = 0
            for bi, b in enumerate(order):
                kb = kbs[b]
                qt, kt, vt = qts[b], kts[b], vts[b]
                wts = [[], []]
                last = bi == len(order) - 1
                rec = {"b": b, "kb": kb, "wts": wts, "vt": vt, "acc": [None] * 2}
                for t in range(kb):
                    # both heads' QK back-to-back: 64-row matmuls at explicit
                    # tile_position (0,0)/(64,0) run concurrently on separate
                    # PE-array row halves.
                    scs = []
                    for h in range(2):
                        scs.append(sc_pool.tile([128, S], F32, tag="sc",
                                                name="sc"))
                    keep_warm(scs[0][:, 0:512], 1)
                    keep_warm(scs[1][:, 0:512], 1)
                    for qc in range(2):
                        for h in range(2):
                            base = 64 * h
                            nc.tensor.matmul(
                                scs[h][:, qc * 512:(qc + 1) * 512],
                                lhsT=kt[base:base + 64, t * 128:(t + 1) * 128],
                                rhs=qt[base:base + 64, qc * 512:(qc + 1) * 512],
                                start=True, stop=True,
                                tile_position=(base, 0),
                            )
                    if last:
                        # head A first so its A-V overlaps head B's exp phase
                        wts[0].append(emit_exp(w_pool, scs[0]))
                        wts[1].append(emit_exp(w_pool, scs[1]))
                        if t == kb - 1:
                            avq.extend((rec, g) for g in range(16))
                    else:
                        for h in range(2):
                            wts[h].append(emit_exp(w_pool, scs[h]))
                    # spread queued A-V groups over the remaining steps
                    step_no += 1
                    rem = max(1, total_steps - step_no)
                    rate = -(-len(avq) // min(rem, 8))
                    for _ in range(min(rate, 6)):
                        if avq:
                            emit_av(*avq.pop(0))
                if not last:
                    avq.extend((rec, g) for g in range(16))

            tj = sc_pool.tile([128, S], F32, tag="sc", name="sctail")
            while avq:
                emit_av(*avq.pop(0))
                keep_warm(tj[:, 0:512], 1)
    nc.compile()
    return nc


_NC_CACHE = {}


def _get_nc(kbs):
    key = tuple(kbs)
    if key not in _NC_CACHE:
        _NC_CACHE[key] = _build_nc(key)
    return _NC_CACHE[key]


def kernel(memory, query, b, seq_len):
    memory = np.asarray(memory)
    query = np.asarray(query)
    bias = np.asarray(b, dtype=np.float32)
    seq_len = np.asarray(seq_len).reshape(-1).astype(np.int64)

    sl = seq_len.copy()
    kbs = [int(min(KT, max(1, -(-int(s) // 128)))) if s > 0 else KT for s in sl]

    # emb[b, k] = exp(bias[k]) * valid; fully-masked batch -> plain softmax
    pos = np.arange(S)[None, :]
    valid = (pos < sl[:, None]) | (sl[:, None] == 0)
    emb = np.exp(bias)[None, :] * valid.astype(np.float32)  # [B, S]

    qh = (query.astype(np.float32) * (DH ** -0.5)).reshape(B, S, H, DH)
    kh = memory[:, :, :UNITS].astype(np.float32).reshape(B, S, H, DH)
    vh = memory[:, :, UNITS:].astype(np.float32).reshape(B, S, H, DH)
    vh = vh * emb[:, :, None, None]  # [B, S, H, DH] value rows pre-masked

    bf = ml_dtypes.bfloat16
    # [B, S, H, DH] -> [B, H, DH, S] transposed layouts
    qTfull = np.ascontiguousarray(qh.transpose(0, 2, 3, 1)).astype(bf)
    kTfull = np.ascontiguousarray(kh.transpose(0, 2, 3, 1)).astype(bf)
    # [B, S, H, DH] -> [B, (t p), H, DH] -> [B, 128, KT, H, DH]
    vtiles = np.ascontiguousarray(
        vh.reshape(B, KT, 128, H, DH).transpose(0, 2, 1, 3, 4)).astype(bf)
    embt = np.ascontiguousarray(
        emb.reshape(B, KT, 128).transpose(0, 2, 1)).astype(bf)  # [B, 128, KT]

    in_maps = []
    for c in range(N_CORES):
        hA, hB = 2 * c, 2 * c + 1
        qT = np.concatenate([qTfull[:, hA], qTfull[:, hB]], axis=1)  # [B,128,S]
        kT = np.concatenate([kTfull[:, hA], kTfull[:, hB]], axis=1)
        vEc = np.empty((B, 128, KT, 130), dtype=bf)
        vEc[..., 0:64] = vtiles[:, :, :, hA, :]
        vEc[..., 64] = embt
        vEc[..., 65:129] = vtiles[:, :, :, hB, :]
        vEc[..., 129] = embt
        in_maps.append({
            "qt": np.ascontiguousarray(qT),
            "kt": np.ascontiguousarray(kT),
            "vt": np.ascontiguousarray(vEc),
        })

    nc = _get_nc(kbs)
    res = run_bass_kernel_spmd(nc, in_maps, core_ids=list(range(N_CORES)))

    out = np.empty((B, S, UNITS), dtype=np.float32)
    for c in range(N_CORES):
        # o [B, 128, 2, 8, 64] -> [B, (j p), (h c)]
        oc = np.asarray(res.results[c]["o"]).astype(np.float32)
        out[:, :, 128 * c:128 * (c + 1)] = oc.transpose(
            0, 3, 1, 2, 4).reshape(B, S, 128)
    return out


# revision 10
# speedup vs baseline: 1.4250x; 1.4250x over previous
"""Sparse-attention Trainium2 kernel (nn_Attention_81398220193933).

Strategy (tensor-parallel over heads, 2 heads per NeuronCore):
  - Host pre-lays-out per-core tensors:
      qT  [B, 128, S]  bf16 : rows 0:64 = headA Q^T / sqrt(dh), rows 64:128 = headB
      kT  [B, 128, S]  bf16 : same for K^T (only the first kb*128 keys loaded)
      vE  [B, 128, KT, 130] bf16 : per k-tile t, partition p = key position
           t*128+p, cols [0]=emb, [1:65]=V_A*emb, [65]=emb, [66:130]=V_B*emb
           where emb[b,k] = exp(bias[k]) * (k < seq_len[b]) (all-valid if
           seq_len==0). Folding the additive key bias + mask multiplicatively
           into V makes the softmax mask/bias free on-device and lets fully
           masked k-tiles be skipped.
  - Device, per batch b and key-tile t (kb = ceil(seq_len/128) tiles):
      scores^T [k=128, q=1024] f32 = K_tile^T.T @ Q^T for both heads; the two
          64-contraction-row matmuls run concurrently on separate PE-array
          row halves (explicit tile_position (0,0)/(64,0)).
      W^T = exp(scores^T): alternates between ScalarE (exact LUT exp -> bf16)
          and VectorE (one-instruction Schraudolph fast-exp: int16(x*128*log2e
          + magic) bitcast to bf16, ~+-2% weight error) to halve the exp wall.
      accT[66, q] += [emb|V]^T.T @ W^T: V is the *stationary* operand (65-col
          weight loads instead of 8x128) and W streams 2x512 columns; row 0
          accumulates the softmax denominator. Accumulated over t in PSUM,
          inline one step behind the QK stream (no drip queue needed).
    Epilogue per (batch, head): one PSUM->SBUF f32 copy (on whichever of
    ScalarE/VectorE is less loaded) + DMA of the transposed numerators and
    denominators; the host does the final divide + transpose (0.2% of FLOPs).
  - HAM keep-warm: junk matmuls write rows 96:128 of the live accumulator
    banks (start=False so the bank's has_written state is untouched), making
    them dependency-free; plus a startup burst during the initial DMA wait.
  - Softmax max-subtraction is unnecessary: logits are O(+-6) and masked keys
    contribute exactly zero through emb; a fully-masked row degenerates to
    softmax over all keys exactly like the jax reference.
  - PSUM: 2 score slots (2 banks each) + 2 head accumulators (2 banks each).
"""

import numpy as np
import ml_dtypes

import concourse.bass as bass
import concourse.mybir as mybir
import concourse.tile as tile
from concourse import bacc
from concourse.bass_utils import run_bass_kernel_spmd

B = 8
S = 1024
UNITS = 1024
H = 16
DH = 64
N_CORES = 8
KT = S // 128  # max key tiles per batch

BF16 = mybir.dt.bfloat16
F32 = mybir.dt.float32
I16 = mybir.dt.int16

# fast-exp: i16 = trunc/round(x * 128*log2e + (16256 - C)); bits viewed as bf16
EXP_SCALE = 128.0 * 1.4426950408889634
EXP_OFF = 16256.0 - 5.25
# engine cost model (us) used only for load balancing between ScalarE/VectorE
ACT_EXP, DVE_EXP = 1.147, 1.192
ACT_CPY, DVE_CPY = 1.0, 1.19


def _build_nc(kbs):
    """Build the SPMD Bass program. kbs: per-batch number of 128-key tiles."""
    nc = bacc.Bacc("TRN2", target_bir_lowering=False, debug=False,
                   num_devices=N_CORES)
    qT = nc.dram_tensor("qt", [B, 128, S], BF16, kind="ExternalInput").ap()
    kT = nc.dram_tensor("kt", [B, 128, S], BF16, kind="ExternalInput").ap()
    vE = nc.dram_tensor("vt", [B, 128, KT, 130], BF16, kind="ExternalInput").ap()
    o = nc.dram_tensor("o", [B, 2, 65, S], F32, kind="ExternalOutput").ap()

    with tile.TileContext(nc) as tc:
        with (
            tc.tile_pool(name="qk", bufs=2) as qk_pool,
            tc.tile_pool(name="v", bufs=2) as v_pool,
            tc.tile_pool(name="w", bufs=8) as w_pool,
            tc.tile_pool(name="ot", bufs=4) as o_pool,
            tc.tile_pool(name="sc", bufs=2, space="PSUM") as sc_pool,
            tc.tile_pool(name="acc", bufs=2, space="PSUM") as acc_pool,
        ):
            bal = {"act": 0.0, "dve": 0.0}

            def emit_exp(sc):
                """exp of one [128, S] score tile on the less-loaded engine."""
                if bal["act"] <= bal["dve"]:
                    bal["act"] += ACT_EXP
                    wt = w_pool.tile([128, S], BF16, tag="w", name="w")
                    nc.scalar.activation(wt[:], sc[:],
                                         mybir.ActivationFunctionType.Exp)
                    return wt
                bal["dve"] += DVE_EXP
                wt = w_pool.tile([128, S], I16, tag="w", name="w")
                nc.vector.tensor_scalar(
                    wt[:], sc[:], EXP_SCALE, EXP_OFF,
                    mybir.AluOpType.mult, mybir.AluOpType.add)
                return wt

            def w_ap(wt):
                ap = wt[:]
                return ap.bitcast(BF16) if wt.tensor.dtype == I16 else ap

            def emit_av(p, t, stop):
                """A-V accumulation for key-tile t of both heads: V stationary
                (65-col weight load), W^T streaming 2x512 columns."""
                for h in range(2):
                    acc = p["acc"][h]
                    wap = w_ap(p["wts"][h][t])
                    for qc in range(2):
                        nc.tensor.matmul(
                            acc[0:65, qc * 512:(qc + 1) * 512],
                            lhsT=p["vt"][:, t, h * 65:h * 65 + 65],
                            rhs=wap[:, qc * 512:(qc + 1) * 512],
                            start=(t == 0), stop=stop,
                        )

            def epilogue(p, h):
                """Copy numerators+denominators PSUM -> SBUF -> HBM."""
                acc = p["acc"][h]
                ot = o_pool.tile([65, S], F32, tag="ot", name="ot")
                if bal["act"] <= bal["dve"]:
                    bal["act"] += ACT_CPY
                    nc.scalar.copy(ot[:], acc[0:65, :])
                else:
                    bal["dve"] += DVE_CPY
                    nc.vector.tensor_copy(ot[:], acc[0:65, :])
                nc.sync.dma_start(out=o[p["b"], h], in_=ot[:])

            # Preload the exp table-set (~2.7us) while the first DMAs fly.
            wexp = qk_pool.tile([1, 8], F32, tag="wexp", name="wexp", bufs=1)
            nc.vector.memset(wexp[:], 0.0)
            nc.scalar.activation(wexp[:], wexp[:],
                                 mybir.ActivationFunctionType.Exp)
            wu = qk_pool.tile([128, 640], BF16, tag="wu", name="wu", bufs=1)
            nc.vector.memset(wu[:], 0.0)

            def keep_warm(out_ap, lhsT, n, start=False):
                for _ in range(n):
                    nc.tensor.matmul(out_ap, lhsT=lhsT, rhs=wu[:, 128:640],
                                     start=start, stop=start,
                                     skip_group_check=True)

            # Load every batch's inputs up front (fits easily in SBUF) so no
            # QK phase ever waits on DMA. First batch small (warms up on real
            # work at low cost), then largest-first, smallest last (short tail
            # after the final exp).
            srt = sorted(range(B), key=lambda i: -kbs[i])
            order = [srt[-2]] + srt[:-2] + [srt[-1]]
            qts, kts, vts = {}, {}, {}
            for b in order:
                qts[b] = qk_pool.tile([128, S], BF16, tag=f"qt{b}",
                                      name=f"qt{b}", bufs=1)
                nc.sync.dma_start(out=qts[b][:], in_=qT[b])
                kts[b] = qk_pool.tile([128, kbs[b] * 128], BF16, tag=f"kt{b}",
                                      name=f"kt{b}", bufs=1)
                nc.sync.dma_start(out=kts[b][:], in_=kT[b, :, :kbs[b] * 128])
            for b in order:
                vts[b] = v_pool.tile([128, kbs[b], 130], BF16, tag=f"vt{b}",
                                     name=f"vt{b}", bufs=1)
                nc.sync.dma_start(out=vts[b][:], in_=vE[b, :, :kbs[b], :])

            # Startup burst: warm the PE during the initial DMA wait.
            scw = sc_pool.tile([128, S], F32, tag="sc", name="scwarm")
            keep_warm(scw[:, 0:512], wu[:, 0:128], 3, start=True)
            keep_warm(scw[:, 512:1024], wu[:, 0:128], 3, start=True)

            prev = None  # batch still owing its last AV + epilogues
            for bi, b in enumerate(order):
                kb = kbs[b]
                qt, kt, vt = qts[b], kts[b], vts[b]
                rec = {"b": b, "kb": kb, "wts": [[], []], "vt": vt,
                       "acc": [None, None]}
                for t in range(kb):
                    # both heads' QK back-to-back at tile_position (0,0) /
                    # (64,0): concurrent on separate PE-array row halves.
                    scs = [sc_pool.tile([128, S], F32, tag="sc", name="sc")
                           for _ in range(2)]
                    # HAM keep-warm pulse: a small junk matmul into the score
                    # slot just before its real QK overwrites it (~107ns).
                    if t > 0 or bi > 0:
                        nc.tensor.matmul(scs[0][:, 0:256], lhsT=wu[:, 0:128],
                                         rhs=wu[:, 128:384],
                                         start=True, stop=True,
                                         skip_group_check=True)
                    for qc in range(2):
                        for h in range(2):
                            base = 64 * h
                            nc.tensor.matmul(
                                scs[h][:, qc * 512:(qc + 1) * 512],
                                lhsT=kt[base:base + 64, t * 128:(t + 1) * 128],
                                rhs=qt[base:base + 64, qc * 512:(qc + 1) * 512],
                                start=True, stop=True,
                                tile_position=(base, 0),
                            )
                    for h in range(2):
                        rec["wts"][h].append(emit_exp(scs[h]))
                    # finish the previous batch: its last AV + epilogues
                    if prev is not None:
                        emit_av(prev, prev["kb"] - 1, stop=True)
                        epilogue(prev, 0)
                        epilogue(prev, 1)
                        prev = None
                    if t == 0:
                        rec["acc"] = [
                            acc_pool.tile([128, S], F32, tag="acc",
                                          name=f"acc{b}_{h}")
                            for h in range(2)]
                    else:
                        emit_av(rec, t - 1, stop=False)
                prev = rec

            emit_av(prev, prev["kb"] - 1, stop=True)
            epilogue(prev, 0)
            epilogue(prev, 1)
    nc.compile()
    return nc


_NC_CACHE = {}


def _get_nc(kbs):
    key = tuple(kbs)
    if key not in _NC_CACHE:
        _NC_CACHE[key] = _build_nc(key)
    return _NC_CACHE[key]


def kernel(memory, query, b, seq_len):
    memory = np.asarray(memory)
    query = np.asarray(query)
    bias = np.asarray(b, dtype=np.float32)
    seq_len = np.asarray(seq_len).reshape(-1).astype(np.int64)

    sl = seq_len.copy()
    kbs = [int(min(KT, max(1, -(-int(s) // 128)))) if s > 0 else KT for s in sl]

    # emb[b, k] = exp(bias[k]) * valid; fully-masked batch -> plain softmax
    pos = np.arange(S)[None, :]
    valid = (pos < sl[:, None]) | (sl[:, None] == 0)
    emb = np.exp(bias)[None, :] * valid.astype(np.float32)  # [B, S]

    qh = (query.astype(np.float32) * (DH ** -0.5)).reshape(B, S, H, DH)
    kh = memory[:, :, :UNITS].astype(np.float32).reshape(B, S, H, DH)
    vh = memory[:, :, UNITS:].astype(np.float32).reshape(B, S, H, DH)
    vh = vh * emb[:, :, None, None]  # [B, S, H, DH] value rows pre-masked

    bf = ml_dtypes.bfloat16
    # [B, S, H, DH] -> [B, H, DH, S] transposed layouts
    qTfull = np.ascontiguousarray(qh.transpose(0, 2, 3, 1)).astype(bf)
    kTfull = np.ascontiguousarray(kh.transpose(0, 2, 3, 1)).astype(bf)
    # [B, S, H, DH] -> [B, (t p), H, DH] -> [B, 128, KT, H, DH]
    vtiles = np.ascontiguousarray(
        vh.reshape(B, KT, 128, H, DH).transpose(0, 2, 1, 3, 4)).astype(bf)
    embt = np.ascontiguousarray(
        emb.reshape(B, KT, 128).transpose(0, 2, 1)).astype(bf)  # [B, 128, KT]

    in_maps = []
    for c in range(N_CORES):
        hA, hB = 2 * c, 2 * c + 1
        qTc = np.concatenate([qTfull[:, hA], qTfull[:, hB]], axis=1)
        kTc = np.concatenate([kTfull[:, hA], kTfull[:, hB]], axis=1)
        vEc = np.empty((B, 128, KT, 130), dtype=bf)
        vEc[..., 0] = embt
        vEc[..., 1:65] = vtiles[:, :, :, hA, :]
        vEc[..., 65] = embt
        vEc[..., 66:130] = vtiles[:, :, :, hB, :]
        in_maps.append({
            "qt": np.ascontiguousarray(qTc),
            "kt": np.ascontiguousarray(kTc),
            "vt": np.ascontiguousarray(vEc),
        })

    nc = _get_nc(kbs)
    res = run_bass_kernel_spmd(nc, in_maps, core_ids=list(range(N_CORES)))

    out = np.empty((B, S, UNITS), dtype=np.float32)
    for c in range(N_CORES):
        # o [B, 2, 65, S]: row 0 = denominators, rows 1:65 = numerators^T
        oc = np.asarray(res.results[c]["o"], dtype=np.float32)
        for h in range(2):
            num = oc[:, h, 1:65, :]            # [B, 64, S]
            den = oc[:, h, 0:1, :]             # [B, 1, S]
            out[:, :, 128 * c + 64 * h:128 * c + 64 * h + 64] = (
                num / den).transpose(0, 2, 1)
    return out
